# revision 1
# baseline (speedup 1.0000x reference)
"""AttentionPooling (segment softmax-pool) Trainium2 kernel.

out[s,:] = sum_n 1[idx[n]==s] * gnorm[n] * (x[n,:] @ msg_w + msg_b)
  gnorm[n] = w[n]^p * exp(gate[n]) / (denom[seg] + eps)   (max-sub skipped:
  mathematically identical after normalization, logits are O(5))

Restructured so the big matmul contracts rows via a one-hot:
  A[s,d]   = sum_n G[n,s] * x[n,d],  denom[s] = sum_n G[n,s]   (ones col)
  out[s,:] = (A[s,:] @ msg_w) / (denom+eps) + (denom/(denom+eps)) * msg_b
where G[n,s] = 1[idx[n]==s] * g[n] is built per 128-row tile with one fused
DVE tensor_scalar(is_equal, mult) against an iota row.

Sharding: index is sorted; host assigns 2048 contiguous segments per core,
16 windows x 128 segments, rows of each window padded to 66*128 = 8448.

Engine assignment (v2): PE = A-matmul + phase2; DVE = G-build, logit reduce,
small ops, phase2 copies; GPSIMD = logit multiply; ACT = exp only (ln hoisted
to one pre-pass) so its LUT never reloads.
"""

import os
import sys
import numpy as np

for _p in ("/opt/trn_rl_repo", "/root/.axon_site/_ro/trn_rl_repo"):
    if os.path.isdir(_p) and _p not in sys.path:
        sys.path.insert(0, _p)

P = 128
S = 16384
D = 128
NCORES = 8
WIN = 64                       # segments per PSUM window
NWIN = S // WIN                # 128 global windows
NWIN_CORE = NWIN // NCORES     # 16 per core
TPW = 34                       # 128-row tiles per window (padded)
GROUP = 17                     # tiles per DMA/logit super-group
GPW = TPW // GROUP             # 6 groups per window
NT = NWIN_CORE * TPW           # 1056 tiles per core
NG = NT // GROUP               # 96 groups per core
ROWS_CORE = NT * P             # 135168 padded rows per core
EPS = 1e-10

IOTA_BF16 = False              # bf16 iota regressed G-build (487 vs 266 ns)
MULT_ON_GPSIMD = False         # gpsimd streaming halves DVE via shared SBUF port
U8_MASK = True                 # host-built u8 one-hot mask kills the is_equal
G_ON_ACT_MOD = 5               # j%5 < 3 -> G-build on ACT (60%); ACT Copy+scale = g*mask
GBUILD_ON_GPSIMD = False       # gpsimd TS measured 2268ns/tile - keep on DVE
ACT_ACCUM_REDUCE = False       # 3D group reduce on DVE hits 2x mode (72ns/tile)

LAST_EXEC_NS = None
LAST_RESULTS = None

_module_cache = {}


def _build_module():
    if "nc" in _module_cache:
        return _module_cache["nc"]

    import concourse.bass as bass  # noqa: F401
    import concourse.tile as tile
    from concourse import bacc, mybir
    from concourse.masks import make_identity

    f32 = mybir.dt.float32
    bf16 = mybir.dt.bfloat16
    iota_dt = bf16 if IOTA_BF16 else f32
    AX = mybir.AxisListType
    ALU = mybir.AluOpType
    ACTF = mybir.ActivationFunctionType

    nc = bacc.Bacc(
        "TRN2",
        target_bir_lowering=False,
        debug=False,
        enable_asserts=True,
        num_devices=NCORES,
    )

    xp = nc.dram_tensor("xp", [NG * P, GROUP * (D + 1)], f32, kind="ExternalInput")
    maskg = nc.dram_tensor(
        "maskg", [NG * P, GROUP * WIN], mybir.dt.uint8, kind="ExternalInput"
    )
    wall = nc.dram_tensor("wall", [P, NT], f32, kind="ExternalInput")
    gwrep = nc.dram_tensor("gwrep", [P, GROUP * D], f32, kind="ExternalInput")
    msgw = nc.dram_tensor("msgw", [D, D], f32, kind="ExternalInput")
    msgbrep = nc.dram_tensor("msgbrep", [P, D], f32, kind="ExternalInput")
    gatebrep = nc.dram_tensor("gatebrep", [P, 1], f32, kind="ExternalInput")
    prep = nc.dram_tensor("prep", [P, 1], f32, kind="ExternalInput")
    out = nc.dram_tensor("out", [NWIN_CORE * WIN, D], f32, kind="ExternalOutput")

    with tile.TileContext(nc) as tc:
        from contextlib import ExitStack

        with ExitStack() as ctx:
            const_pool = ctx.enter_context(tc.tile_pool(name="const", bufs=1))
            xs_pool = ctx.enter_context(tc.tile_pool(name="xs", bufs=10))
            grp_pool = ctx.enter_context(tc.tile_pool(name="grp", bufs=6))
            g_pool = ctx.enter_context(tc.tile_pool(name="gm", bufs=10))
            psA_pool = ctx.enter_context(tc.tile_pool(name="psA", bufs=4, space="PSUM"))
            ps2_pool = ctx.enter_context(tc.tile_pool(name="ps2", bufs=2, space="PSUM"))
            ph2_pool = ctx.enter_context(tc.tile_pool(name="ph2", bufs=3))

            gw_t = const_pool.tile([P, GROUP * D], f32)
            nc.sync.dma_start(gw_t[:], gwrep[:, :])
            msgw_t = const_pool.tile([D, D], f32)
            nc.sync.dma_start(msgw_t[:], msgw[:, :])
            msgb_t = const_pool.tile([P, D], f32)
            nc.sync.dma_start(msgb_t[:], msgbrep[:, :])
            gateb_t = const_pool.tile([P, 1], f32)
            nc.sync.dma_start(gateb_t[:], gatebrep[:, :])
            p_t = const_pool.tile([P, 1], f32)
            nc.sync.dma_start(p_t[:], prep[:, :])
            ident = const_pool.tile([P, P], f32)
            make_identity(nc, ident[:])

            # hoisted: p*ln(w) for every tile in two ops
            w_t = const_pool.tile([P, NT], f32)
            nc.sync.dma_start(w_t[:], wall[:, :])
            plw_t = const_pool.tile([P, NT], f32)
            nc.scalar.activation(out=plw_t[:], in_=w_t[:], func=ACTF.Ln)
            nc.vector.tensor_scalar_mul(plw_t[:], plw_t[:], p_t[:, 0:1])

            gw3 = gw_t[:].rearrange("p (t d) -> p t d", d=D)

            # software pipeline: emit group g+1's logit chain before group g's
            # G-builds so exp(g+1) lands ahead of the G(g) ops in ACT's stream
            chains = {}

            def emit_chain(g):
                xs = xs_pool.tile([P, GROUP * (D + 1)], f32, tag="xs", name=f"xs{g}")
                nc.sync.dma_start(xs[:], xp[g * P : (g + 1) * P, :])
                xs3 = xs[:].rearrange("p (t d) -> p t d", d=D + 1)
                mk = xs_pool.tile(
                    [P, GROUP * WIN], mybir.dt.uint8, tag="mk", name=f"mk{g}"
                )
                nc.sync.dma_start(mk[:], maskg[g * P : (g + 1) * P, :])
                xw = grp_pool.tile([P, GROUP * D], f32, tag="xw", name=f"xw{g}")
                xw3 = xw[:].rearrange("p (t d) -> p t d", d=D)
                nc.vector.tensor_tensor(
                    out=xw3, in0=xs3[:, :, 0:D], in1=gw3, op=ALU.mult
                )
                logit = grp_pool.tile([P, GROUP], f32, tag="logit", name=f"lg{g}")
                nc.vector.reduce_sum(out=logit[:], in_=xw3, axis=AX.X)
                logit2 = grp_pool.tile([P, GROUP], f32, tag="logit2", name=f"l2{g}")
                nc.vector.tensor_add(
                    logit2[:], logit[:], plw_t[:, g * GROUP : (g + 1) * GROUP]
                )
                gex = grp_pool.tile([P, GROUP], f32, tag="gex", name=f"gx{g}")
                nc.scalar.activation(
                    out=gex[:], in_=logit2[:], func=ACTF.Exp, bias=gateb_t[:, 0:1]
                )
                chains[g] = (xs3, mk, gex)

            def emit_gmm(g, psA):
                xs3, mk, gex = chains.pop(g)
                gi = g % GPW
                for j in range(GROUP):
                    t_in_win = gi * GROUP + j
                    t_glob = g * GROUP + j
                    G = g_pool.tile([P, WIN], f32, tag="G", name=f"G{t_glob}")
                    if t_glob % 4 < 3:
                        nc.scalar.activation(
                            out=G[:],
                            in_=mk[:, j * WIN : (j + 1) * WIN],
                            func=ACTF.Copy,
                            scale=gex[:, j : j + 1],
                        )
                    else:
                        nc.vector.tensor_scalar(
                            out=G[:],
                            in0=mk[:, j * WIN : (j + 1) * WIN],
                            scalar1=gex[:, j : j + 1],
                            scalar2=None,
                            op0=ALU.mult,
                        )
                    nc.tensor.matmul(
                        out=psA[:],
                        lhsT=G[:],
                        rhs=xs3[:, j, :],
                        start=(t_in_win == 0),
                        stop=(t_in_win == TPW - 1),
                    )

            def emit_phase2(w, psA):
                sbA = ph2_pool.tile([WIN, D + 1], f32, tag="sbA", name=f"sbA{w}")
                nc.vector.tensor_copy(sbA[:], psA[:])
                deno = ph2_pool.tile([WIN, 1], f32, tag="deno", name=f"dn{w}")
                nc.vector.tensor_scalar_add(deno[:], sbA[:, D : D + 1], EPS)
                rcp = ph2_pool.tile([WIN, 1], f32, tag="rcp", name=f"rc{w}")
                nc.vector.reciprocal(out=rcp[:], in_=deno[:])
                coef = ph2_pool.tile([WIN, 1], f32, tag="coef", name=f"cf{w}")
                nc.vector.tensor_tensor(
                    out=coef[:], in0=sbA[:, D : D + 1], in1=rcp[:], op=ALU.mult
                )
                psAT = ps2_pool.tile([P, WIN], f32, tag="AT", name=f"AT{w}")
                nc.tensor.transpose(
                    out=psAT[:], in_=sbA[:, 0:D], identity=ident[:WIN, :WIN]
                )
                sbAT = ph2_pool.tile([P, WIN], f32, tag="sbAT", name=f"sT{w}")
                nc.vector.tensor_copy(sbAT[:], psAT[:])
                ps2 = ps2_pool.tile([WIN, D], f32, tag="out2", name=f"o2{w}")
                nc.tensor.matmul(
                    out=ps2[:], lhsT=sbAT[:], rhs=msgw_t[:], start=True, stop=True
                )
                outsb = ph2_pool.tile([WIN, D], f32, tag="outsb", name=f"ou{w}")
                nc.scalar.activation(
                    out=outsb[:], in_=ps2[:], func=ACTF.Copy, scale=rcp[:, 0:1]
                )
                bterm = ph2_pool.tile([WIN, D], f32, tag="bterm", name=f"bt{w}")
                nc.scalar.activation(
                    out=bterm[:], in_=msgb_t[:WIN, :], func=ACTF.Copy,
                    scale=coef[:, 0:1],
                )
                ofin = ph2_pool.tile([WIN, D], f32, tag="ofin", name=f"of{w}")
                nc.vector.tensor_add(ofin[:], outsb[:], bterm[:])
                nc.sync.dma_start(out[w * WIN : (w + 1) * WIN, :], ofin[:])

            psA_tiles = {}
            for g in range(NG):
                emit_chain(g)
                w = g // GPW
                if g % GPW == 0:
                    psA_tiles[w] = psA_pool.tile(
                        [WIN, D + 1], f32, tag="psA", name=f"psA{w}"
                    )
                emit_gmm(g, psA_tiles[w])
                if g % GPW == GPW - 1:
                    emit_phase2(w, psA_tiles.pop(w))

    nc.compile()
    _module_cache["nc"] = nc
    return nc


def _shard_inputs(x, idx, w):
    """Pad + reorder host arrays into the per-core device layouts."""
    n = idx.shape[0]
    bounds = np.searchsorted(idx, np.arange(0, S + 1, WIN)).astype(np.int64)
    counts = np.diff(bounds)
    if counts.max() > TPW * P:
        raise RuntimeError(f"window overflow: {counts.max()} > {TPW * P}")

    dest = np.arange(n, dtype=np.int64) + np.repeat(
        np.arange(NWIN, dtype=np.int64) * (TPW * P) - bounds[:-1], counts
    )

    xpad = np.zeros((NCORES * ROWS_CORE, D + 1), dtype=np.float32)
    xpad[:, D] = 1.0
    xpad[dest, 0:D] = x
    idxl = np.zeros(NCORES * ROWS_CORE, dtype=np.float32)
    idxl[dest] = (idx - np.repeat(np.arange(NWIN, dtype=np.int64) * WIN, counts)).astype(
        np.float32
    )
    wpad = np.ones(NCORES * ROWS_CORE, dtype=np.float32)
    wpad[dest] = w

    # device layout: per core, per group: [128 partitions, GROUP tiles, ...]
    xdev = (
        xpad.reshape(NCORES, NG, GROUP, P, D + 1)
        .transpose(0, 1, 3, 2, 4)
        .reshape(NCORES, NG * P, GROUP * (D + 1))
    )
    mask = np.zeros((NCORES * ROWS_CORE, WIN), dtype=np.uint8)
    mask[dest, idxl[dest].astype(np.int64)] = 1
    maskdev = (
        mask.reshape(NCORES, NG, GROUP, P, WIN)
        .transpose(0, 1, 3, 2, 4)
        .reshape(NCORES, NG * P, GROUP * WIN)
    )
    wdev = np.ascontiguousarray(wpad.reshape(NCORES, NT, P).transpose(0, 2, 1))
    return xdev, maskdev, wdev


def _ensure_ntff_hook():
    """The image's antenv package lacks axon_hooks; shim it so trace=True
    can register the ctypes NTFF hook from trn_agent_boot."""
    try:
        from antenv.axon_hooks import get_axon_ntff_profile_hook  # noqa: F401

        return True
    except ImportError:
        pass
    try:
        import types

        import antenv
        from trn_agent_boot.trn_boot import _ntff_profile_via_ctypes

        mod = types.ModuleType("antenv.axon_hooks")
        _hook = [None]
        mod.set_axon_ntff_profile_hook = lambda h: _hook.__setitem__(0, h)
        mod.get_axon_ntff_profile_hook = lambda: _hook[0]
        sys.modules["antenv.axon_hooks"] = mod
        antenv.axon_hooks = mod
        mod.set_axon_ntff_profile_hook(
            _ntff_profile_via_ctypes("/opt/axon/libaxon_pjrt.so")
        )
        return True
    except Exception as e:  # degrade to untraced run
        print(f"ntff hook install failed: {type(e).__name__}: {e}")
        return False


def kernel(x, index, weights, gate_w, gate_b, msg_w, msg_b, pow_p):
    global LAST_EXEC_NS, LAST_RESULTS

    x = np.ascontiguousarray(np.asarray(x, dtype=np.float32))
    idx = np.asarray(index).astype(np.int64).ravel()
    w = np.asarray(weights, dtype=np.float32).ravel()
    gate_w = np.asarray(gate_w, dtype=np.float32).reshape(D)
    gate_b = np.asarray(gate_b, dtype=np.float32).reshape(1)
    msg_w = np.ascontiguousarray(np.asarray(msg_w, dtype=np.float32))
    msg_b = np.asarray(msg_b, dtype=np.float32).reshape(D)
    pow_p = np.asarray(pow_p, dtype=np.float32).reshape(1)

    if not np.all(idx[1:] >= idx[:-1]):
        perm = np.argsort(idx, kind="stable")
        idx = idx[perm]
        x = x[perm]
        w = w[perm]

    xdev, maskdev, wdev = _shard_inputs(x, idx, w)

    gwrep = np.tile(gate_w[None, :], (P, GROUP)).astype(np.float32)
    msgbrep = np.tile(msg_b[None, :], (P, 1)).astype(np.float32)
    gatebrep = np.full((P, 1), gate_b[0], dtype=np.float32)
    prep = np.full((P, 1), pow_p[0], dtype=np.float32)
    nc = _build_module()
    from concourse.bass_utils import run_bass_kernel_spmd

    in_maps = []
    for c in range(NCORES):
        in_maps.append(
            {
                "xp": np.ascontiguousarray(xdev[c]),
                "maskg": np.ascontiguousarray(maskdev[c]),
                "wall": wdev[c],
                "gwrep": gwrep,
                "msgw": msg_w,
                "msgbrep": msgbrep,
                "gatebrep": gatebrep,
                "prep": prep,
            }
        )

    trace = bool(os.environ.get("KERNEL_TRACE"))
    if trace:
        trace = _ensure_ntff_hook()
    res = run_bass_kernel_spmd(
        nc, in_maps, core_ids=list(range(NCORES)), trace=trace
    )
    LAST_RESULTS = res
    LAST_EXEC_NS = res.exec_time_ns

    out = np.concatenate([res.results[c]["out"] for c in range(NCORES)], axis=0)
    return out.astype(np.float32)


def kernel_numpy(x, index, weights, gate_w, gate_b, msg_w, msg_b, pow_p):
    """Host-side mirror of the device algorithm (debug only)."""
    x = np.asarray(x, dtype=np.float32)
    idx = np.asarray(index).astype(np.int64).ravel()
    w = np.asarray(weights, dtype=np.float32).ravel()
    gate = x @ np.asarray(gate_w, dtype=np.float32).reshape(D, 1)
    gate = gate[:, 0] + np.asarray(gate_b).reshape(1)[0]
    g = np.exp(gate + np.asarray(pow_p).reshape(1)[0] * np.log(w))
    A = np.zeros((S, D), dtype=np.float64)
    den = np.zeros(S, dtype=np.float64)
    np.add.at(A, idx, g[:, None] * x)
    np.add.at(den, idx, g)
    out = (A @ np.asarray(msg_w, dtype=np.float64)) / (den[:, None] + EPS)
    out = out + (den / (den + EPS))[:, None] * np.asarray(msg_b).reshape(1, D)
    return out.astype(np.float32)



# revision 11
# speedup vs baseline: 1.6042x; 1.6042x over previous
"""AttentionPooling (segment softmax-pool) Trainium2 kernel, v3.

out[s,:] = sum_n 1[idx[n]==s] * gnorm[n] * (x[n,:] @ msg_w + msg_b)
  gnorm[n] = w[n]^p * exp(gate[n]) / (denom[seg] + eps)   (max-sub skipped:
  mathematically identical after normalization, logits are O(5))

v3 restructure vs the f32 baseline (647us -> target <250us):
  * everything bf16: PE matmuls 1 cyc/row (vs 4 for f32), DVE 2x/4x modes,
    half the HBM traffic.  rel tolerance is 2e-2; bf16 lands ~1e-3.
  * gate_w folded into x on the host: device sees xg = x * gate_w and
    msg_w' = msg_w / gate_w (exact identity: A@msg_w == (A*gw)@(msg_w/gw),
    denominator column unscaled).  The per-row gate logit then needs NO
    multiply on device -- just a reduction, done as a 7-level binary add
    tree (tensor_tensor has a 2x mode; tensor_reduce has none).  Levels
    1-2 are bf16 (the bulk of the work), levels 3-7 f32: an all-bf16
    tree costs 1.1e-2 rel err (vs the 2e-2 gate), this mix 4.6e-3.
  * one-hot G built mask-free per 128-row tile with a single fused DVE
    tensor_scalar(is_equal, mult) vs a bf16 iota row: 4x DVE mode.
  * exact per-window tiling: module is specialized (and cached) to the
    actual per-window tile counts (max over cores per window slot),
    ~1019 tiles/core vs 1088 padded.

Phase 1 per 128-row tile t of window w (64 segments per window):
  G[p,s]    = (iota[s] == idxl[p,t]) * gex[p,t]          (DVE, 4x)
  psA[s,c] += sum_p G[p,s] * xg[p,t,c]                   (PE, c = 0..128,
              col 128 is the ones column -> denominators)
Phase 2 per window: numer = (psA[:,0:128])^T-major matmul with msg_w',
  out = numer * (1/(denom+eps)) (+ msg_b term only if msg_b != 0).
"""

import os
import sys
import numpy as np

for _p in ("/opt/trn_rl_repo", "/root/.axon_site/_ro/trn_rl_repo"):
    if os.path.isdir(_p) and _p not in sys.path:
        sys.path.insert(0, _p)

P = 128
S = 16384
D = 128
NCORES = 8
WIN = 64                       # segments per PSUM window
NWIN = S // WIN                # 256 global windows
NWC = NWIN // NCORES           # 32 windows per core
EPS = 1e-10
SENT = 999.0                   # idxl sentinel for padded rows -> G row = 0

LAST_EXEC_NS = None
LAST_RESULTS = None

_module_cache = {}


def _build_module(tpw, has_bias):
    """tpw: tuple of NWC per-window tile counts (uniform across cores)."""
    key = (tpw, has_bias)
    if key in _module_cache:
        return _module_cache[key]

    import concourse.bass as bass  # noqa: F401
    import concourse.tile as tile
    from concourse import bacc, mybir

    f32 = mybir.dt.float32
    bf16 = mybir.dt.bfloat16
    ALU = mybir.AluOpType
    ACTF = mybir.ActivationFunctionType

    NT = int(sum(tpw))
    tbase = [0]
    for t in tpw:
        tbase.append(tbase[-1] + t)

    nc = bacc.Bacc(
        "TRN2",
        target_bir_lowering=False,
        debug=False,
        enable_asserts=True,
        num_devices=NCORES,
    )

    xp = nc.dram_tensor("xp", [P, NT * (D + 1)], bf16, kind="ExternalInput")
    idxla = nc.dram_tensor("idxla", [P, NT], f32, kind="ExternalInput")
    wall = nc.dram_tensor("wall", [P, NT], f32, kind="ExternalInput")
    iota = nc.dram_tensor("iota", [P, WIN], bf16, kind="ExternalInput")
    identb = nc.dram_tensor("identb", [WIN, WIN], f32, kind="ExternalInput")
    msgwp = nc.dram_tensor("msgwp", [D, D], f32, kind="ExternalInput")
    gatebrep = nc.dram_tensor("gatebrep", [P, 1], f32, kind="ExternalInput")
    prep = nc.dram_tensor("prep", [P, 1], f32, kind="ExternalInput")
    if has_bias:
        msgbrep = nc.dram_tensor("msgbrep", [P, D], f32, kind="ExternalInput")
    out = nc.dram_tensor("out", [NWC * WIN, D], f32, kind="ExternalOutput")

    with tile.TileContext(nc) as tc:
        from contextlib import ExitStack

        with ExitStack() as ctx:
            const_pool = ctx.enter_context(tc.tile_pool(name="const", bufs=1))
            xs_pool = ctx.enter_context(tc.tile_pool(name="xs", bufs=4))
            tr_pool = ctx.enter_context(tc.tile_pool(name="tr", bufs=2))
            lg_pool = ctx.enter_context(tc.tile_pool(name="lg", bufs=3))
            g_pool = ctx.enter_context(tc.tile_pool(name="gm", bufs=8))
            psA_pool = ctx.enter_context(tc.tile_pool(name="psA", bufs=3, space="PSUM"))
            psT_pool = ctx.enter_context(tc.tile_pool(name="psT", bufs=2, space="PSUM"))
            ps2_pool = ctx.enter_context(tc.tile_pool(name="ps2", bufs=2, space="PSUM"))
            ph_pool = ctx.enter_context(tc.tile_pool(name="ph", bufs=3))

            iota_t = const_pool.tile([P, WIN], bf16)
            nc.sync.dma_start(iota_t[:], iota[:, :])
            ident_t = const_pool.tile([WIN, WIN], f32)
            nc.sync.dma_start(ident_t[:], identb[:, :])
            msgw_t = const_pool.tile([D, D], f32)
            nc.sync.dma_start(msgw_t[:], msgwp[:, :])
            gateb_t = const_pool.tile([P, 1], f32)
            nc.sync.dma_start(gateb_t[:], gatebrep[:, :])
            p_t = const_pool.tile([P, 1], f32)
            nc.sync.dma_start(p_t[:], prep[:, :])
            idxl_t = const_pool.tile([P, NT], f32)
            nc.sync.dma_start(idxl_t[:], idxla[:, :])
            if has_bias:
                msgb_t = const_pool.tile([P, D], f32)
                nc.sync.dma_start(msgb_t[:], msgbrep[:, :])

            # hoisted: plw = pow_p * ln(w) for every tile in two ops
            w_t = const_pool.tile([P, NT], f32)
            nc.sync.dma_start(w_t[:], wall[:, :])
            plw_t = const_pool.tile([P, NT], f32)
            nc.scalar.activation(out=plw_t[:], in_=w_t[:], func=ACTF.Ln)
            nc.vector.tensor_scalar_mul(plw_t[:], plw_t[:], p_t[:, 0:1])

            chains = {}

            def emit_logits(w):
                T = tpw[w]
                base = tbase[w]
                xs = xs_pool.tile([P, T * (D + 1)], bf16, tag="xs", name=f"xs{w}")
                nc.sync.dma_start(
                    xs[:], xp[:, base * (D + 1) : (base + T) * (D + 1)]
                )
                xs3 = xs[:].rearrange("p (t c) -> p t c", c=D + 1)
                rA = tr_pool.tile([P, T * 64], bf16, tag="rA", name=f"rA{w}")
                rA3 = rA[:].rearrange("p (t c) -> p t c", c=64)
                rB = tr_pool.tile([P, T * 32], bf16, tag="rB", name=f"rB{w}")
                rB3 = rB[:].rearrange("p (t c) -> p t c", c=32)
                rC = tr_pool.tile([P, T * 16], f32, tag="rC", name=f"rC{w}")
                rC3 = rC[:].rearrange("p (t c) -> p t c", c=16)
                rD = tr_pool.tile([P, T * 8], f32, tag="rD", name=f"rD{w}")
                rD3 = rD[:].rearrange("p (t c) -> p t c", c=8)
                TT = nc.vector.tensor_tensor
                TT(out=rA3, in0=xs3[:, :, 0:64], in1=xs3[:, :, 64:128], op=ALU.add)
                TT(out=rB3, in0=rA3[:, :, 0:32], in1=rA3[:, :, 32:64], op=ALU.add)
                TT(out=rC3, in0=rB3[:, :, 0:16], in1=rB3[:, :, 16:32], op=ALU.add)
                TT(out=rD3, in0=rC3[:, :, 0:8], in1=rC3[:, :, 8:16], op=ALU.add)
                TT(out=rC3[:, :, 0:4], in0=rD3[:, :, 0:4], in1=rD3[:, :, 4:8],
                   op=ALU.add)
                TT(out=rD3[:, :, 0:2], in0=rC3[:, :, 0:2], in1=rC3[:, :, 2:4],
                   op=ALU.add)
                lg = lg_pool.tile([P, T], f32, tag="lg", name=f"lg{w}")
                lg3 = lg[:].rearrange("p (t c) -> p t c", c=1)
                TT(out=lg3, in0=rD3[:, :, 0:1], in1=rD3[:, :, 1:2], op=ALU.add)
                lg2 = lg_pool.tile([P, T], f32, tag="lg2", name=f"lh{w}")
                nc.vector.tensor_add(lg2[:], lg[:], plw_t[:, base : base + T])
                gex = lg_pool.tile([P, T], f32, tag="gex", name=f"gx{w}")
                nc.scalar.activation(
                    out=gex[:], in_=lg2[:], func=ACTF.Exp, bias=gateb_t[:, 0:1]
                )
                chains[w] = (xs3, gex)

            def emit_gmm(w, psA):
                T = tpw[w]
                base = tbase[w]
                xs3, gex = chains.pop(w)
                for j in range(T):
                    tg = base + j
                    G = g_pool.tile([P, WIN], bf16, tag="G", name=f"G{tg}")
                    nc.vector.tensor_scalar(
                        out=G[:],
                        in0=iota_t[:],
                        scalar1=idxl_t[:, tg : tg + 1],
                        scalar2=gex[:, j : j + 1],
                        op0=ALU.is_equal,
                        op1=ALU.mult,
                    )
                    nc.tensor.matmul(
                        out=psA[:],
                        lhsT=G[:],
                        rhs=xs3[:, j, :],
                        start=(j == 0),
                        stop=(j == T - 1),
                    )

            def emit_phase2(w, psA):
                sbA = ph_pool.tile([WIN, D + 1], f32, tag="sbA", name=f"sbA{w}")
                nc.vector.tensor_copy(sbA[:], psA[:])
                deno = ph_pool.tile([WIN, 1], f32, tag="deno", name=f"dn{w}")
                nc.vector.tensor_scalar_add(deno[:], psA[:, D : D + 1], EPS)
                rcp = ph_pool.tile([WIN, 1], f32, tag="rcp", name=f"rc{w}")
                nc.vector.reciprocal(out=rcp[:], in_=deno[:])
                psAT = psT_pool.tile([P, WIN], f32, tag="AT", name=f"AT{w}")
                nc.tensor.transpose(
                    out=psAT[:], in_=sbA[:, 0:D], identity=ident_t[:, :]
                )
                sbAT = ph_pool.tile([P, WIN], f32, tag="sbAT", name=f"sT{w}")
                nc.vector.tensor_copy(sbAT[:], psAT[:])
                ps2 = ps2_pool.tile([WIN, D], f32, tag="out2", name=f"o2{w}")
                nc.tensor.matmul(
                    out=ps2[:], lhsT=sbAT[:], rhs=msgw_t[:], start=True, stop=True
                )
                outsb = ph_pool.tile([WIN, D], f32, tag="outsb", name=f"ou{w}")
                nc.scalar.activation(
                    out=outsb[:], in_=ps2[:], func=ACTF.Copy, scale=rcp[:, 0:1]
                )
                fin = outsb
                if has_bias:
                    coef = ph_pool.tile([WIN, 1], f32, tag="coef", name=f"cf{w}")
                    nc.vector.tensor_tensor(
                        out=coef[:], in0=psA[:, D : D + 1], in1=rcp[:], op=ALU.mult
                    )
                    bt = ph_pool.tile([WIN, D], f32, tag="bt", name=f"bt{w}")
                    nc.scalar.activation(
                        out=bt[:], in_=msgb_t[:WIN, :], func=ACTF.Copy,
                        scale=coef[:, 0:1],
                    )
                    fin = ph_pool.tile([WIN, D], f32, tag="fin", name=f"fi{w}")
                    nc.vector.tensor_add(fin[:], outsb[:], bt[:])
                nc.sync.dma_start(out[w * WIN : (w + 1) * WIN, :], fin[:])

            # software pipeline: window w+1's logit chain is emitted before
            # window w's G-builds so DVE never stalls on ACT's exp.
            emit_logits(0)
            for w in range(NWC):
                if w + 1 < NWC:
                    emit_logits(w + 1)
                psA = psA_pool.tile([WIN, D + 1], f32, tag="psA", name=f"psA{w}")
                emit_gmm(w, psA)
                emit_phase2(w, psA)

    nc.compile()
    _module_cache[key] = nc
    return nc


def _layout(idx):
    bounds = np.searchsorted(idx, np.arange(0, S + 1, WIN)).astype(np.int64)
    counts = np.diff(bounds)
    tiles = -(-counts // P)
    tpw = tiles.reshape(NCORES, NWC).max(axis=0)
    tbase = np.concatenate([[0], np.cumsum(tpw)]).astype(np.int64)
    return bounds, counts, tpw, tbase


def _shard_inputs(x, idx, w, gwc, bounds, counts, tpw, tbase):
    """Pad + reorder host arrays into the per-core device layouts."""
    from concourse import mybir

    bf16 = mybir.dt.np(mybir.dt.bfloat16)
    n = idx.shape[0]
    NT = int(tbase[-1])

    wg = np.repeat(np.arange(NWIN, dtype=np.int64), counts)
    k = np.arange(n, dtype=np.int64) - np.repeat(bounds[:-1], counts)
    w_slot = wg % NWC
    core = wg // NWC
    flat = (tbase[w_slot] + k // P) * P + (k % P)
    rowpos = core * (NT * P) + flat

    xall = np.zeros((NCORES * NT * P, D + 1), dtype=np.float32)
    xall[rowpos, 0:D] = x * gwc[None, :]
    xall[rowpos, D] = 1.0

    idxl = np.full(NCORES * NT * P, SENT, dtype=np.float32)
    idxl[rowpos] = (idx - wg * WIN).astype(np.float32)
    wpad = np.ones(NCORES * NT * P, dtype=np.float32)
    wpad[rowpos] = w

    # device layout per core: [P, NT*(D+1)] bf16, tile-major columns
    xdev = []
    for c in range(NCORES):
        xc = xall[c * NT * P : (c + 1) * NT * P].reshape(NT, P, D + 1)
        xdev.append(
            np.ascontiguousarray(xc.transpose(1, 0, 2)).reshape(P, NT * (D + 1))
            .astype(bf16)
        )
    idxldev = np.ascontiguousarray(
        idxl.reshape(NCORES, NT, P).transpose(0, 2, 1)
    )
    wdev = np.ascontiguousarray(wpad.reshape(NCORES, NT, P).transpose(0, 2, 1))
    return xdev, idxldev, wdev


def _ensure_ntff_hook():
    """The image's antenv package lacks axon_hooks; shim it so trace=True
    can register the ctypes NTFF hook from trn_agent_boot."""
    try:
        from antenv.axon_hooks import get_axon_ntff_profile_hook  # noqa: F401

        return True
    except ImportError:
        pass
    try:
        import types

        import antenv
        from trn_agent_boot.trn_boot import _ntff_profile_via_ctypes

        mod = types.ModuleType("antenv.axon_hooks")
        _hook = [None]
        mod.set_axon_ntff_profile_hook = lambda h: _hook.__setitem__(0, h)
        mod.get_axon_ntff_profile_hook = lambda: _hook[0]
        sys.modules["antenv.axon_hooks"] = mod
        antenv.axon_hooks = mod
        mod.set_axon_ntff_profile_hook(
            _ntff_profile_via_ctypes("/opt/axon/libaxon_pjrt.so")
        )
        return True
    except Exception as e:  # degrade to untraced run
        print(f"ntff hook install failed: {type(e).__name__}: {e}")
        return False


def kernel(x, index, weights, gate_w, gate_b, msg_w, msg_b, pow_p):
    global LAST_EXEC_NS, LAST_RESULTS
    from concourse import mybir

    bf16 = mybir.dt.np(mybir.dt.bfloat16)

    x = np.ascontiguousarray(np.asarray(x, dtype=np.float32))
    idx = np.asarray(index).astype(np.int64).ravel()
    w = np.asarray(weights, dtype=np.float32).ravel()
    gate_w = np.asarray(gate_w, dtype=np.float32).reshape(D)
    gate_b = np.asarray(gate_b, dtype=np.float32).reshape(1)
    msg_w = np.ascontiguousarray(np.asarray(msg_w, dtype=np.float32))
    msg_b = np.asarray(msg_b, dtype=np.float32).reshape(D)
    pow_p = np.asarray(pow_p, dtype=np.float32).reshape(1)

    if not np.all(idx[1:] >= idx[:-1]):
        perm = np.argsort(idx, kind="stable")
        idx = idx[perm]
        x = x[perm]
        w = w[perm]

    # fold gate_w into x; un-fold via msg_w' = msg_w / gwc (exact identity)
    gwc = np.where(np.abs(gate_w) < 1e-6,
                   np.where(gate_w < 0, -1e-6, 1e-6), gate_w).astype(np.float32)
    msgwp = (msg_w / gwc[:, None]).astype(np.float32)

    bounds, counts, tpw, tbase = _layout(idx)
    has_bias = bool(np.any(msg_b != 0.0))
    nc = _build_module(tuple(int(t) for t in tpw), has_bias)

    xdev, idxldev, wdev = _shard_inputs(x, idx, w, gwc, bounds, counts, tpw, tbase)

    iota = np.tile(np.arange(WIN, dtype=np.float32)[None, :], (P, 1)).astype(bf16)
    identb = np.eye(WIN, dtype=np.float32)
    gatebrep = np.full((P, 1), gate_b[0], dtype=np.float32)
    prep = np.full((P, 1), pow_p[0], dtype=np.float32)

    from concourse.bass_utils import run_bass_kernel_spmd

    in_maps = []
    for c in range(NCORES):
        m = {
            "xp": xdev[c],
            "idxla": idxldev[c],
            "wall": wdev[c],
            "iota": iota,
            "identb": identb,
            "msgwp": msgwp,
            "gatebrep": gatebrep,
            "prep": prep,
        }
        if has_bias:
            m["msgbrep"] = np.tile(msg_b[None, :], (P, 1)).astype(np.float32)
        in_maps.append(m)

    trace = bool(os.environ.get("KERNEL_TRACE"))
    if trace:
        trace = _ensure_ntff_hook()
    res = run_bass_kernel_spmd(
        nc, in_maps, core_ids=list(range(NCORES)), trace=trace
    )
    LAST_RESULTS = res
    LAST_EXEC_NS = res.exec_time_ns

    out = np.concatenate([res.results[c]["out"] for c in range(NCORES)], axis=0)
    return out.astype(np.float32)


def kernel_numpy(x, index, weights, gate_w, gate_b, msg_w, msg_b, pow_p):
    """Host-side mirror of the device algorithm with bf16 rounding (debug)."""
    try:
        import ml_dtypes
        bf16 = ml_dtypes.bfloat16
    except ImportError:
        from concourse import mybir
        bf16 = mybir.dt.np(mybir.dt.bfloat16)

    x = np.asarray(x, dtype=np.float32)
    idx = np.asarray(index).astype(np.int64).ravel()
    w = np.asarray(weights, dtype=np.float32).ravel()
    gw = np.asarray(gate_w, dtype=np.float32).reshape(D)
    gwc = np.where(np.abs(gw) < 1e-6, np.where(gw < 0, -1e-6, 1e-6), gw)
    xg = (x * gwc[None, :]).astype(bf16)
    # binary-tree logit reduce: levels 1-2 bf16, rest f32 (mirrors device)
    t = xg.astype(np.float32)
    width = D
    lvl = 0
    while width > 1:
        width //= 2
        lvl += 1
        t = t[:, 0:width] + t[:, width : 2 * width]
        if lvl <= 2:
            t = t.astype(bf16).astype(np.float32)
    logit = t[:, 0]
    g = np.exp(
        logit
        + np.asarray(pow_p).reshape(1)[0] * np.log(w)
        + np.asarray(gate_b).reshape(1)[0]
    ).astype(np.float32)
    A = np.zeros((S, D), dtype=np.float64)
    den = np.zeros(S, dtype=np.float64)
    gb = g.astype(bf16).astype(np.float64)
    np.add.at(A, idx, gb[:, None] * xg.astype(np.float64))
    np.add.at(den, idx, gb)
    Ab = A.astype(np.float32).astype(np.float64)
    Wb = (np.asarray(msg_w, np.float32) / gwc[:, None]).astype(np.float64)
    out = (Ab @ Wb) / (den[:, None] + EPS)
    out = out + (den / (den + EPS))[:, None] * np.asarray(msg_b).reshape(1, D)
    return out.astype(np.float32)


# revision 20
# speedup vs baseline: 2.6574x; 1.6565x over previous
"""AttentionPooling (segment softmax-pool) Trainium2 kernel, v3.

out[s,:] = sum_n 1[idx[n]==s] * gnorm[n] * (x[n,:] @ msg_w + msg_b)
  gnorm[n] = w[n]^p * exp(gate[n]) / (denom[seg] + eps)   (max-sub skipped:
  mathematically identical after normalization, logits are O(5))

v3 restructure vs the f32 baseline (647us -> target <250us):
  * everything bf16: PE matmuls 1 cyc/row (vs 4 for f32), DVE 2x/4x modes,
    half the HBM traffic.  rel tolerance is 2e-2; bf16 lands ~1e-3.
  * gate_w folded into x on the host: device sees xg = x * gate_w and
    msg_w' = msg_w / gate_w (exact identity: A@msg_w == (A*gw)@(msg_w/gw),
    denominator column unscaled).  The per-row gate logit then needs NO
    multiply on device -- just a reduction, done as a 7-level binary add
    tree (tensor_tensor has a 2x mode; tensor_reduce has none).  Levels
    1-2 are bf16 (the bulk of the work), levels 3-7 f32: an all-bf16
    tree costs 1.1e-2 rel err (vs the 2e-2 gate), this mix 4.6e-3.
  * one-hot G built per WINDOW (not per tile): host supplies a u8 one-hot
    mask strip [128, T*64]; one DVE tensor_tensor multiply against a
    stride-0 broadcast of gex[p,t] builds the whole window's G strip.
    (per-tile tensor_scalar builds cost ~207ns/op of fixed overhead;
    per-window strips amortize it 30x.)
  * exact per-window tiling: module is specialized (and cached) to the
    actual per-window tile counts (max over cores per window slot),
    ~1019 tiles/core vs 1088 padded.

Phase 1 per 128-row tile t of window w (64 segments per window):
  G[p,s]    = (iota[s] == idxl[p,t]) * gex[p,t]          (DVE, 4x)
  psA[s,c] += sum_p G[p,s] * xg[p,t,c]                   (PE, c = 0..128,
              col 128 is the ones column -> denominators)
Phase 2 per window: numer = (psA[:,0:128])^T-major matmul with msg_w',
  out = numer * (1/(denom+eps)) (+ msg_b term only if msg_b != 0).
"""

import os
import sys
import numpy as np

for _p in ("/opt/trn_rl_repo", "/root/.axon_site/_ro/trn_rl_repo"):
    if os.path.isdir(_p) and _p not in sys.path:
        sys.path.insert(0, _p)

P = 128
S = 16384
D = 128
NCORES = 8
WIN = 64                       # segments per PSUM window
NWIN = S // WIN                # 256 global windows
NWC = NWIN // NCORES           # 32 windows per core
EPS = 1e-10
SENT = 999.0                   # idxl sentinel for padded rows -> G row = 0

LAST_EXEC_NS = None
LAST_RESULTS = None

_module_cache = {}


def _build_module(tpw, has_bias):
    """tpw: tuple of NWC per-window tile counts (uniform across cores)."""
    key = (tpw, has_bias)
    if key in _module_cache:
        return _module_cache[key]

    import concourse.bass as bass  # noqa: F401
    import concourse.tile as tile
    from concourse import bacc, mybir

    f32 = mybir.dt.float32
    bf16 = mybir.dt.bfloat16
    ALU = mybir.AluOpType
    ACTF = mybir.ActivationFunctionType

    NT = int(sum(tpw))
    tbase = [0]
    for t in tpw:
        tbase.append(tbase[-1] + t)

    nc = bacc.Bacc(
        "TRN2",
        target_bir_lowering=False,
        debug=False,
        enable_asserts=True,
        num_devices=NCORES,
    )

    xp = nc.dram_tensor("xp", [P, NT * (D + 1)], bf16, kind="ExternalInput")
    maskg = nc.dram_tensor("maskg", [P, NT * WIN], mybir.dt.uint8,
                           kind="ExternalInput")
    wall = nc.dram_tensor("wall", [P, NT], f32, kind="ExternalInput")
    identb = nc.dram_tensor("identb", [WIN, WIN], f32, kind="ExternalInput")
    msgwp = nc.dram_tensor("msgwp", [D, D], f32, kind="ExternalInput")
    gatebrep = nc.dram_tensor("gatebrep", [P, 1], f32, kind="ExternalInput")
    prep = nc.dram_tensor("prep", [P, 1], f32, kind="ExternalInput")
    if has_bias:
        msgbrep = nc.dram_tensor("msgbrep", [P, D], f32, kind="ExternalInput")
    out = nc.dram_tensor("out", [NWC * WIN, D], f32, kind="ExternalOutput")

    with tile.TileContext(nc) as tc:
        from contextlib import ExitStack

        with ExitStack() as ctx:
            const_pool = ctx.enter_context(tc.tile_pool(name="const", bufs=1))
            xs_pool = ctx.enter_context(tc.tile_pool(name="xs", bufs=4))
            tr_pool = ctx.enter_context(tc.tile_pool(name="tr", bufs=2))
            lg_pool = ctx.enter_context(tc.tile_pool(name="lg", bufs=3))
            g_pool = ctx.enter_context(tc.tile_pool(name="gm", bufs=8))
            psA_pool = ctx.enter_context(tc.tile_pool(name="psA", bufs=3, space="PSUM"))
            psT_pool = ctx.enter_context(tc.tile_pool(name="psT", bufs=2, space="PSUM"))
            ps2_pool = ctx.enter_context(tc.tile_pool(name="ps2", bufs=2, space="PSUM"))
            ph_pool = ctx.enter_context(tc.tile_pool(name="ph", bufs=3))

            ident_t = const_pool.tile([WIN, WIN], f32)
            nc.sync.dma_start(ident_t[:], identb[:, :])
            msgw_t = const_pool.tile([D, D], f32)
            nc.sync.dma_start(msgw_t[:], msgwp[:, :])
            gateb_t = const_pool.tile([P, 1], f32)
            nc.sync.dma_start(gateb_t[:], gatebrep[:, :])
            p_t = const_pool.tile([P, 1], f32)
            nc.sync.dma_start(p_t[:], prep[:, :])
            if has_bias:
                msgb_t = const_pool.tile([P, D], f32)
                nc.sync.dma_start(msgb_t[:], msgbrep[:, :])

            # hoisted: plw = pow_p * ln(w) for every tile in two ops
            w_t = const_pool.tile([P, NT], f32)
            nc.sync.dma_start(w_t[:], wall[:, :])
            plw_t = const_pool.tile([P, NT], f32)
            nc.scalar.activation(out=plw_t[:], in_=w_t[:], func=ACTF.Ln)
            nc.vector.tensor_scalar_mul(plw_t[:], plw_t[:], p_t[:, 0:1])

            chains = {}

            def emit_logits(w):
                T = tpw[w]
                base = tbase[w]
                xs = xs_pool.tile([P, T * (D + 1)], bf16, tag="xs", name=f"xs{w}")
                nc.sync.dma_start(
                    xs[:], xp[:, base * (D + 1) : (base + T) * (D + 1)]
                )
                xs3 = xs[:].rearrange("p (t c) -> p t c", c=D + 1)
                mk = xs_pool.tile([P, T * WIN], mybir.dt.uint8, tag="mk",
                                  name=f"mk{w}")
                nc.sync.dma_start(mk[:], maskg[:, base * WIN : (base + T) * WIN])
                rA = tr_pool.tile([P, T * 64], bf16, tag="rA", name=f"rA{w}")
                rA3 = rA[:].rearrange("p (t c) -> p t c", c=64)
                rB = tr_pool.tile([P, T * 32], bf16, tag="rB", name=f"rB{w}")
                rB3 = rB[:].rearrange("p (t c) -> p t c", c=32)
                rC = tr_pool.tile([P, T * 16], bf16, tag="rC", name=f"rC{w}")
                rC3 = rC[:].rearrange("p (t c) -> p t c", c=16)
                rD = tr_pool.tile([P, T * 8], f32, tag="rD", name=f"rD{w}")
                rD3 = rD[:].rearrange("p (t c) -> p t c", c=8)
                rE = tr_pool.tile([P, T * 4], f32, tag="rE", name=f"rE{w}")
                rE3 = rE[:].rearrange("p (t c) -> p t c", c=4)
                TT = nc.vector.tensor_tensor
                TT(out=rA3, in0=xs3[:, :, 0:64], in1=xs3[:, :, 64:128], op=ALU.add)
                TT(out=rB3, in0=rA3[:, :, 0:32], in1=rA3[:, :, 32:64], op=ALU.add)
                TT(out=rC3, in0=rB3[:, :, 0:16], in1=rB3[:, :, 16:32], op=ALU.add)
                TT(out=rD3, in0=rC3[:, :, 0:8], in1=rC3[:, :, 8:16], op=ALU.add)
                TT(out=rE3, in0=rD3[:, :, 0:4], in1=rD3[:, :, 4:8], op=ALU.add)
                TT(out=rD3[:, :, 0:2], in0=rE3[:, :, 0:2], in1=rE3[:, :, 2:4],
                   op=ALU.add)
                lg = lg_pool.tile([P, T], f32, tag="lg", name=f"lg{w}")
                lg3 = lg[:].rearrange("p (t c) -> p t c", c=1)
                TT(out=lg3, in0=rD3[:, :, 0:1], in1=rD3[:, :, 1:2], op=ALU.add)
                lg2 = lg_pool.tile([P, T], f32, tag="lg2", name=f"lh{w}")
                nc.vector.tensor_add(lg2[:], lg[:], plw_t[:, base : base + T])
                gex = lg_pool.tile([P, T], bf16, tag="gex", name=f"gx{w}")
                nc.scalar.activation(
                    out=gex[:], in_=lg2[:], func=ACTF.Exp, bias=gateb_t[:, 0:1]
                )
                chains[w] = (xs3, mk, gex)

            def emit_gmm(w, psA):
                T = tpw[w]
                xs3, mk, gex = chains.pop(w)
                mk3 = mk[:].rearrange("p (t s) -> p t s", s=WIN)
                Gs = g_pool.tile([P, T * WIN], bf16, tag="G", name=f"G{w}")
                G3 = Gs[:].rearrange("p (t s) -> p t s", s=WIN)
                gexb = gex[:].unsqueeze(2).to_broadcast((P, T, WIN))
                nc.vector.tensor_tensor(out=G3, in0=mk3, in1=gexb, op=ALU.mult)
                for j in range(T):
                    nc.tensor.matmul(
                        out=psA[:],
                        lhsT=G3[:, j, :],
                        rhs=xs3[:, j, :],
                        start=(j == 0),
                        stop=(j == T - 1),
                    )

            def emit_phase2(w, psA):
                sbA = ph_pool.tile([WIN, D + 1], f32, tag="sbA", name=f"sbA{w}")
                nc.scalar.activation(out=sbA[:], in_=psA[:], func=ACTF.Copy)
                deno = ph_pool.tile([WIN, 1], f32, tag="deno", name=f"dn{w}")
                nc.scalar.activation(
                    out=deno[:], in_=psA[:, D : D + 1], func=ACTF.Copy, bias=EPS
                )
                rcp = ph_pool.tile([WIN, 1], f32, tag="rcp", name=f"rc{w}")
                nc.vector.reciprocal(out=rcp[:], in_=deno[:])
                psAT = psT_pool.tile([P, WIN], f32, tag="AT", name=f"AT{w}")
                nc.tensor.transpose(
                    out=psAT[:], in_=sbA[:, 0:D], identity=ident_t[:, :]
                )
                sbAT = ph_pool.tile([P, WIN], f32, tag="sbAT", name=f"sT{w}")
                nc.scalar.activation(out=sbAT[:], in_=psAT[:], func=ACTF.Copy)
                ps2 = ps2_pool.tile([WIN, D], f32, tag="out2", name=f"o2{w}")
                nc.tensor.matmul(
                    out=ps2[:], lhsT=sbAT[:], rhs=msgw_t[:], start=True, stop=True
                )
                outsb = ph_pool.tile([WIN, D], f32, tag="outsb", name=f"ou{w}")
                nc.scalar.activation(
                    out=outsb[:], in_=ps2[:], func=ACTF.Copy, scale=rcp[:, 0:1]
                )
                fin = outsb
                if has_bias:
                    coef = ph_pool.tile([WIN, 1], f32, tag="coef", name=f"cf{w}")
                    nc.vector.tensor_tensor(
                        out=coef[:], in0=psA[:, D : D + 1], in1=rcp[:], op=ALU.mult
                    )
                    bt = ph_pool.tile([WIN, D], f32, tag="bt", name=f"bt{w}")
                    nc.scalar.activation(
                        out=bt[:], in_=msgb_t[:WIN, :], func=ACTF.Copy,
                        scale=coef[:, 0:1],
                    )
                    fin = ph_pool.tile([WIN, D], f32, tag="fin", name=f"fi{w}")
                    nc.vector.tensor_add(fin[:], outsb[:], bt[:])
                nc.sync.dma_start(out[w * WIN : (w + 1) * WIN, :], fin[:])

            # software pipeline: window w+1's logit chain is emitted before
            # window w's G-builds so DVE never stalls on ACT's exp.
            emit_logits(0)
            for w in range(NWC):
                if w + 1 < NWC:
                    emit_logits(w + 1)
                psA = psA_pool.tile([WIN, D + 1], f32, tag="psA", name=f"psA{w}")
                emit_gmm(w, psA)
                emit_phase2(w, psA)

    nc.compile()
    _module_cache[key] = nc
    return nc


def _layout(idx):
    bounds = np.searchsorted(idx, np.arange(0, S + 1, WIN)).astype(np.int64)
    counts = np.diff(bounds)
    tiles = -(-counts // P)
    tpw = tiles.reshape(NCORES, NWC).max(axis=0)
    tbase = np.concatenate([[0], np.cumsum(tpw)]).astype(np.int64)
    return bounds, counts, tpw, tbase


def _shard_inputs(x, idx, w, gwc, bounds, counts, tpw, tbase):
    """Pad + reorder host arrays into the per-core device layouts."""
    from concourse import mybir

    bf16 = mybir.dt.np(mybir.dt.bfloat16)
    n = idx.shape[0]
    NT = int(tbase[-1])

    wg = np.repeat(np.arange(NWIN, dtype=np.int64), counts)
    k = np.arange(n, dtype=np.int64) - np.repeat(bounds[:-1], counts)
    w_slot = wg % NWC
    core = wg // NWC
    flat = (tbase[w_slot] + k // P) * P + (k % P)
    rowpos = core * (NT * P) + flat

    xall = np.zeros((NCORES * NT * P, D + 1), dtype=np.float32)
    xall[rowpos, 0:D] = x * gwc[None, :]
    xall[rowpos, D] = 1.0

    mask = np.zeros((NCORES * NT * P, WIN), dtype=np.uint8)
    mask[rowpos, (idx - wg * WIN).astype(np.int64)] = 1
    wpad = np.ones(NCORES * NT * P, dtype=np.float32)
    wpad[rowpos] = w

    # device layout per core: [P, NT*(D+1)] bf16, tile-major columns
    xdev, maskdev = [], []
    for c in range(NCORES):
        xc = xall[c * NT * P : (c + 1) * NT * P].reshape(NT, P, D + 1)
        xdev.append(
            np.ascontiguousarray(xc.transpose(1, 0, 2)).reshape(P, NT * (D + 1))
            .astype(bf16)
        )
        mc = mask[c * NT * P : (c + 1) * NT * P].reshape(NT, P, WIN)
        maskdev.append(
            np.ascontiguousarray(mc.transpose(1, 0, 2)).reshape(P, NT * WIN)
        )
    wdev = np.ascontiguousarray(wpad.reshape(NCORES, NT, P).transpose(0, 2, 1))
    return xdev, maskdev, wdev


def _ensure_ntff_hook():
    """The image's antenv package lacks axon_hooks; shim it so trace=True
    can register the ctypes NTFF hook from trn_agent_boot."""
    try:
        from antenv.axon_hooks import get_axon_ntff_profile_hook  # noqa: F401

        return True
    except ImportError:
        pass
    try:
        import types

        import antenv
        from trn_agent_boot.trn_boot import _ntff_profile_via_ctypes

        mod = types.ModuleType("antenv.axon_hooks")
        _hook = [None]
        mod.set_axon_ntff_profile_hook = lambda h: _hook.__setitem__(0, h)
        mod.get_axon_ntff_profile_hook = lambda: _hook[0]
        sys.modules["antenv.axon_hooks"] = mod
        antenv.axon_hooks = mod
        mod.set_axon_ntff_profile_hook(
            _ntff_profile_via_ctypes("/opt/axon/libaxon_pjrt.so")
        )
        return True
    except Exception as e:  # degrade to untraced run
        print(f"ntff hook install failed: {type(e).__name__}: {e}")
        return False


def kernel(x, index, weights, gate_w, gate_b, msg_w, msg_b, pow_p):
    global LAST_EXEC_NS, LAST_RESULTS
    from concourse import mybir

    bf16 = mybir.dt.np(mybir.dt.bfloat16)

    x = np.ascontiguousarray(np.asarray(x, dtype=np.float32))
    idx = np.asarray(index).astype(np.int64).ravel()
    w = np.asarray(weights, dtype=np.float32).ravel()
    gate_w = np.asarray(gate_w, dtype=np.float32).reshape(D)
    gate_b = np.asarray(gate_b, dtype=np.float32).reshape(1)
    msg_w = np.ascontiguousarray(np.asarray(msg_w, dtype=np.float32))
    msg_b = np.asarray(msg_b, dtype=np.float32).reshape(D)
    pow_p = np.asarray(pow_p, dtype=np.float32).reshape(1)

    if not np.all(idx[1:] >= idx[:-1]):
        perm = np.argsort(idx, kind="stable")
        idx = idx[perm]
        x = x[perm]
        w = w[perm]

    # fold gate_w into x; un-fold via msg_w' = msg_w / gwc (exact identity)
    gwc = np.where(np.abs(gate_w) < 1e-6,
                   np.where(gate_w < 0, -1e-6, 1e-6), gate_w).astype(np.float32)
    msgwp = (msg_w / gwc[:, None]).astype(np.float32)

    bounds, counts, tpw, tbase = _layout(idx)
    has_bias = bool(np.any(msg_b != 0.0))
    nc = _build_module(tuple(int(t) for t in tpw), has_bias)

    xdev, maskdev, wdev = _shard_inputs(x, idx, w, gwc, bounds, counts, tpw, tbase)

    identb = np.eye(WIN, dtype=np.float32)
    gatebrep = np.full((P, 1), gate_b[0], dtype=np.float32)
    prep = np.full((P, 1), pow_p[0], dtype=np.float32)

    from concourse.bass_utils import run_bass_kernel_spmd

    in_maps = []
    for c in range(NCORES):
        m = {
            "xp": xdev[c],
            "maskg": maskdev[c],
            "wall": wdev[c],
            "identb": identb,
            "msgwp": msgwp,
            "gatebrep": gatebrep,
            "prep": prep,
        }
        if has_bias:
            m["msgbrep"] = np.tile(msg_b[None, :], (P, 1)).astype(np.float32)
        in_maps.append(m)

    trace = bool(os.environ.get("KERNEL_TRACE"))
    if trace:
        trace = _ensure_ntff_hook()
    res = run_bass_kernel_spmd(
        nc, in_maps, core_ids=list(range(NCORES)), trace=trace
    )
    LAST_RESULTS = res
    LAST_EXEC_NS = res.exec_time_ns

    out = np.concatenate([res.results[c]["out"] for c in range(NCORES)], axis=0)
    return out.astype(np.float32)


def kernel_numpy(x, index, weights, gate_w, gate_b, msg_w, msg_b, pow_p):
    """Host-side mirror of the device algorithm with bf16 rounding (debug)."""
    try:
        import ml_dtypes
        bf16 = ml_dtypes.bfloat16
    except ImportError:
        from concourse import mybir
        bf16 = mybir.dt.np(mybir.dt.bfloat16)

    x = np.asarray(x, dtype=np.float32)
    idx = np.asarray(index).astype(np.int64).ravel()
    w = np.asarray(weights, dtype=np.float32).ravel()
    gw = np.asarray(gate_w, dtype=np.float32).reshape(D)
    gwc = np.where(np.abs(gw) < 1e-6, np.where(gw < 0, -1e-6, 1e-6), gw)
    xg = (x * gwc[None, :]).astype(bf16)
    # binary-tree logit reduce: levels 1-2 bf16, rest f32 (mirrors device)
    t = xg.astype(np.float32)
    width = D
    lvl = 0
    while width > 1:
        width //= 2
        lvl += 1
        t = t[:, 0:width] + t[:, width : 2 * width]
        if lvl <= 3:
            t = t.astype(bf16).astype(np.float32)
    logit = t[:, 0]
    g = np.exp(
        logit
        + np.asarray(pow_p).reshape(1)[0] * np.log(w)
        + np.asarray(gate_b).reshape(1)[0]
    ).astype(np.float32)
    A = np.zeros((S, D), dtype=np.float64)
    den = np.zeros(S, dtype=np.float64)
    gb = g.astype(bf16).astype(np.float64)
    np.add.at(A, idx, gb[:, None] * xg.astype(np.float64))
    np.add.at(den, idx, gb)
    Ab = A.astype(np.float32).astype(np.float64)
    Wb = (np.asarray(msg_w, np.float32) / gwc[:, None]).astype(np.float64)
    out = (Ab @ Wb) / (den[:, None] + EPS)
    out = out + (den / (den + EPS))[:, None] * np.asarray(msg_b).reshape(1, D)
    return out.astype(np.float32)
